# revision 52
# baseline (speedup 1.0000x reference)
"""Causal self-attention Bass kernel for 8 TRN2 NeuronCores.

Problem: B=4, T=2048, C=1024, H=16 heads, head_dim=64, fp32.
    q = x @ Wq.T ; k = x @ Wk.T ; v = x @ Wv.T          (per head)
    att = softmax(mask(q k^T / 8))
    y = att @ v ; out = y @ Wp.T

Sharding (8 cores): 4-way data parallel over batch x 2-way tensor
parallel over heads. Core c handles batch c//2 and heads 8*(c%2)..+8.
Wq/Wk/Wv column-parallel, Wp row-parallel; the partial outputs of the
two head-halves of each batch are summed on the host (the "all-reduce"
of row-parallel Wp).

Pipeline design (v7):
  - Projections stream xT by 512-wide t-chunks (tch).  The causal
    structure means attention q-chunk qc only needs projections from
    chunks <= qc, so attention blocks INTERLEAVE with projection
    blocks: tch0, tch1, qc0, tch2, qc1, tch3, qc2, qc3.  The
    projection blocks are PE-bound while attention is ACT/DVE-heavy,
    so interleaving keeps every engine fed.
  - exp alternates between ACT (even kt, true exp) and DVE (odd kt,
    Schraudolph bf16 bit-hack: bits = round(s*23.083 + 16248.5) as
    int16 reinterpreted bf16; ~1.8% rms on half the weights, ~5e-3
    added rel err after softmax averaging).
  - PV lags exp by 3 kts; scores/PV emitted in 2-kt batches to halve
    rhs-stream-switch pipeline refills.
  - Diagonal kts run FIRST within each (qc, pr) so the serial
    exp->mask(gpsimd)->PV chain overlaps the pr-start bubble and the
    pr tail drains mask-free.
  - Output projection chains interleave into later attention blocks.
  - Softmax normalize: sums (ones-column of the PV stationary) ->
    ACT/DVE copies release the psum accumulators early -> DVE
    reciprocal -> DRAM-bounce broadcast -> gpsimd scale (deferred one
    pr so gpsimd's FIFO doesn't delay causal masks).
  - Everything bf16: FWL keeps LDWEIGHTS off the critical path, input
    DMA traffic halves.  fp32 accumulation throughout.
"""

from contextlib import ExitStack

import numpy as np

import concourse.bass as bass
import concourse.tile as tile
from concourse import bacc, mybir

F32 = mybir.dt.float32
BF16 = mybir.dt.bfloat16
I16 = mybir.dt.int16

B, T, C, H, D = 4, 2048, 1024, 16, 64
NCORES = 8
JL = 512            # local j dims per core (8 heads * 64)
NPAIR = 4           # local head pairs
CI = C // 128       # 8 c-tiles
NT = T // 128       # 16 t/k tiles
NQC = T // 512      # 4 q chunks
VW = D + 1

# Schraudolph bf16 bit-hack exp: bits = s*0.125*log2(e)*128 + (127*128 - 7.5)
EXP_A = 0.125 * 1.4426950408889634 * 128.0
EXP_B = 127.0 * 128.0 - 7.5

_CACHED_NC = None


def build_nc():
    nc = bacc.Bacc(None)

    xT = nc.dram_tensor("xT", [C, T], BF16, kind="ExternalInput")
    wqT = nc.dram_tensor("wqT", [C, JL], BF16, kind="ExternalInput")
    wkT = nc.dram_tensor("wkT", [C, JL], BF16, kind="ExternalInput")
    wvT = nc.dram_tensor("wvT", [C, JL], BF16, kind="ExternalInput")
    wpT = nc.dram_tensor("wpT", [JL, C], BF16, kind="ExternalInput")
    out = nc.dram_tensor("out", [T, C], BF16, kind="ExternalOutput")
    # bounce buffer for broadcasting softmax reciprocals across partitions
    rcd = nc.dram_tensor("rcd", [NPAIR, NQC, 2, 512], F32)

    xT_r = xT.rearrange("(ci p) t -> p ci t", p=128)
    wq_r = wqT.rearrange("(ci p) j -> p ci j", p=128)
    wk_r = wkT.rearrange("(ci p) j -> p ci j", p=128)
    wv_r = wvT.rearrange("(ci p) j -> p ci j", p=128)
    wp_r = wpT.rearrange("(ji p) c -> p ji c", p=128)

    with tile.TileContext(nc) as tc, ExitStack() as ctx:
        pm = ctx.enter_context(tc.tile_pool(name="pm", bufs=1))
        qkp = ctx.enter_context(tc.tile_pool(name="qkp", bufs=1))
        expp = ctx.enter_context(tc.tile_pool(name="expp", bufs=6))
        bcp = ctx.enter_context(tc.tile_pool(name="bcp", bufs=4))
        # two (sL, sH) f32 staging tiles per normalize, held one pr deep by
        # the deferred scale -> 4 bufs
        sab = ctx.enter_context(tc.tile_pool(name="sab", bufs=4))
        outp = ctx.enter_context(tc.tile_pool(name="outp", bufs=3))
        wpool = ctx.enter_context(tc.tile_pool(name="ph1w", bufs=1))
        xpool = ctx.enter_context(tc.tile_pool(name="ph1x", bufs=2))
        # PSUM: scores (f32, 2 banks each) x3 + shared f32 ring x2 = 8 banks
        gp = ctx.enter_context(tc.tile_pool(name="gp", bufs=3, space="PSUM"))
        psf = ctx.enter_context(tc.tile_pool(name="psf", bufs=2, space="PSUM"))

        # HAM warm-up: the PE clock-gate defaults to 1.2 GHz and only
        # un-throttles after ~3.4us of sustained activity.  The first real
        # matmul can't start until its weights+x DMA lands (~11us), so spend
        # the DMA wait on dummy back-to-back matmuls -- by the time the
        # first projection chain issues, the PE runs at 2.4 GHz.
        warm = pm.tile([128, 128], BF16)
        nc.vector.memset(warm[:], 0.0)
        wacc = psf.tile([128, 512], F32, tag="ps", name="wacc")

        def emit_dummies(n):
            for i in range(n):
                nc.tensor.matmul(
                    wacc[:, 0:128], warm[:], warm[:],
                    start=(i == 0), stop=(i == n - 1),
                )

        emit_dummies(0)

        # v storage per head-pair: two 128-col stationary blocks
        #   lo: [ v_lo(d 0..63) | ones | 0(x63) ]
        #   hi: [ 0(x32) | ones | 0(x31) | v_hi(d 0..63) ]
        # The lo PV matmul puts y_lo on psum partitions 0-63 and sums_lo on
        # partition 64; the hi matmul puts sums_hi on partition 32 and y_hi
        # on partitions 64-127.  With the two heads' outputs on DISJOINT
        # partition halves (and sums on 32-aligned partitions), the
        # psum->SBUF copies and the normalize multiply are partition-
        # aligned, so the scaled output is written straight into the yT
        # slab by ACT/DVE/gpsimd -- no DMA partition-shift bounce.
        # all-ones column tile: rows 64 and 32 serve as the K=1 stationary
        # of the final normalize's broadcast matmuls
        ones_pe = pm.tile([128, 64], BF16)
        nc.vector.memset(ones_pe[:], 1.0)

        VS = 256
        v_sb = pm.tile([128, NT, NPAIR * VS], BF16)
        v_r = v_sb.rearrange("p n (pr c) -> p n pr c", c=VS)
        nc.vector.memset(v_r[:, :, :, 64:192], 0.0)
        nc.vector.memset(v_r[:, :, :, 64:65], 1.0)
        nc.vector.memset(v_r[:, :, :, 160:161], 1.0)

        qT_all = qkp.tile([128, NPAIR, T], BF16, tag="qT_all")
        kT_all = qkp.tile([128, NPAIR, T], BF16, tag="kT_all")
        # one yT tile per head-pair: precise dependency tracking, so an
        # out-proj chain's ji<3 matmuls never wait on a fresher pr's yT
        yTs = [
            qkp.tile([128, T], BF16, tag=f"yT{j}", name=f"yT{j}")
            for j in range(NPAIR)
        ]
        wp_sb = qkp.tile([128, NPAIR, C], BF16, tag="wp")

        wq_sb = wpool.tile([128, CI, JL], BF16, tag="wq")
        wk_sb = wpool.tile([128, CI, JL], BF16, tag="wk")
        wv_sb = wpool.tile([128, CI, JL], BF16, tag="wv")
        # input DMAs in strict consumption order, one stream per HW DGE.
        # The DMA engines fair-share descriptors across all in-flight
        # transfers, so a non-urgent transfer issued early steals bandwidth
        # from the urgent ones.  Queue issue alone does NOT serialize
        # (it's just descriptor enqueue) -- force ordering with tiny fence
        # DMAs that read the tail of the previous transfer and write the
        # head of the next one's buffer (RAW + WAW deps stall the queue
        # until the previous transfer completes).
        xt0 = xpool.tile([128, CI, 512], BF16, tag="xt")
        q = 2
        nc.sync.dma_start(wq_sb[:, 0:q, :], wq_r[:, 0:q, :])
        nc.scalar.dma_start(xt0[:, 0:q, :], xT_r[:, 0:q, 0:512])
        nc.sync.dma_start(wq_sb[:, q:CI, :], wq_r[:, q:CI, :])
        nc.scalar.dma_start(xt0[:, q:CI, :], xT_r[:, q:CI, 0:512])
        nc.sync.dma_start(wk_sb[:, 0, 0:2], wq_sb[:, CI - 1, 510:512])
        nc.sync.dma_start(wk_sb[:], wk_r[:])
        nc.scalar.dma_start(wv_sb[:, 0, 0:2], xt0[:, CI - 1, 510:512])
        nc.scalar.dma_start(wv_sb[:], wv_r[:])
        # wp is issued after xt1's load (fenced on it) -- it isn't needed
        # until the first out-projection (~t+45us) and would otherwise
        # steal DMA bandwidth from xt1 during the startup crunch

        # ---- projection chains for one 512-wide t-chunk -------------------
        def proj_chains(tch, xt, first_mid=None):
            ts_ = slice(tch * 512, tch * 512 + 512)
            chains = []

            def qk_chain(w_sb, dst, eng, pr, mid=None):
                def emit():
                    acc = psf.tile([128, 512], F32, tag="ps", name="pacc")
                    for ci in range(CI):
                        nc.tensor.matmul(
                            acc[:],
                            w_sb[:, ci, pr * 128 : pr * 128 + 128],
                            xt[:, ci, :],
                            start=(ci == 0),
                            stop=(ci == CI - 1),
                        )
                        if mid is not None and ci == 1:
                            # keep the PE clock-gate warm across the
                            # startup DMA stall (ci2..7 land ~4us later)
                            mid()
                    if eng == "v":
                        nc.vector.tensor_copy(dst[:, pr, ts_], acc[:])
                    else:
                        nc.scalar.copy(dst[:, pr, ts_], acc[:])
                return emit

            def v_chain(tl):
                def emit():
                    ti = tch * 4 + tl
                    acc = psf.tile([128, 512], F32, tag="ps", name="pacc")
                    for ci in range(CI):
                        nc.tensor.matmul(
                            acc[:],
                            xt[:, ci, tl * 128 : tl * 128 + 128],
                            wv_sb[:, ci, :],
                            start=(ci == 0),
                            stop=(ci == CI - 1),
                        )
                    ar = acc[:].rearrange(
                        "p (pr two d) -> p pr two d", two=2, d=D
                    )
                    nc.vector.tensor_copy(
                        v_r[:, ti, :, 0:64], ar[:, :, 0, :]
                    )
                    nc.vector.tensor_copy(
                        v_r[:, ti, :, 192:256], ar[:, :, 1, :]
                    )
                return emit

            for pr in range(NPAIR):
                chains.append(
                    qk_chain(
                        wq_sb, qT_all, "v", pr,
                        mid=first_mid if pr == 0 else None,
                    )
                )
            for pr in range(NPAIR):
                chains.append(qk_chain(wk_sb, kT_all, "a", pr))
            for tl in range(4):
                chains.append(v_chain(tl))
            return chains

        def proj_block(tch, xt, first_mid=None):
            for emit in proj_chains(tch, xt, first_mid=first_mid):
                emit()

        # ---- attention machinery -----------------------------------------
        outq = []          # pending out-proj (ti, cc) chains
        n_chain = [0]
        o2_tiles = {}      # ti -> [128, 1024] staging tile (cc=0 allocates)

        def emit_outproj(ti, cc, drain=False):
            tss = slice(ti * 128, ti * 128 + 128)
            cs = slice(cc * 512, cc * 512 + 512)
            acc2 = gp.tile([128, 2, 512], F32, tag="g")
            acc = acc2[:, 0, :]
            for ji in range(NPAIR):
                nc.tensor.matmul(
                    acc,
                    yTs[ji][:, tss],
                    wp_sb[:, ji, cs],
                    start=(ji == 0),
                    stop=(ji == NPAIR - 1),
                )
            # stage both cc halves of a ti into one [128, 1024] tile so the
            # out DMA moves 2KB-contiguous rows (efficient descriptors)
            if ti not in o2_tiles:
                o2_tiles[ti] = outp.tile([128, C], BF16, tag="o", name=f"o2_{ti}")
            o2 = o2_tiles[ti]
            if n_chain[0] % 2 == 0:
                nc.vector.tensor_copy(o2[:, cs], acc)
            else:
                nc.scalar.copy(o2[:, cs], acc)
            n_chain[0] += 1
            if cc == 1:
                # split rows across DGEs so the transfer drains in parallel;
                # the final drain also enlists gpsimd's SWDGE
                r0 = ti * 128
                if drain:
                    nc.sync.dma_start(out[r0 : r0 + 48, :], o2[0:48, :])
                    nc.scalar.dma_start(out[r0 + 48 : r0 + 96, :], o2[48:96, :])
                    nc.gpsimd.dma_start(out[r0 + 96 : r0 + 128, :], o2[96:128, :])
                else:
                    # mid-kernel: keep the issue off ACT (it carries exp +
                    # copies); the sync DGE has headroom here
                    nc.sync.dma_start(out[r0 : r0 + 64, :], o2[0:64, :])
                    nc.sync.dma_start(out[r0 + 64 : r0 + 128, :], o2[64:128, :])
                del o2_tiles[ti]

        # deferred normalize tail: the gpsimd scale-multiplies of (qc, pr)
        # are emitted after (qc, pr+1)'s masks so gpsimd's strict FIFO
        # doesn't delay the causal-mask affine_selects.  The muls write the
        # scaled attention output DIRECTLY into the yT slab (partition-
        # aligned thanks to the v layout) -- no DMA bounce.
        deferred = []

        def pop_deferred(eng=None):
            # lo half on gpsimd (partition base 0 -- the only base gpsimd
            # handles reliably), hi half on DVE
            sLd, sHd, bcd, pr_, qs_ = deferred.pop(0)
            (eng or nc.gpsimd).tensor_mul(
                yTs[pr_][0:64, qs_], sLd[0:64, :], bcd[0:64, :]
            )
            (eng or nc.gpsimd).tensor_mul(
                yTs[pr_][64:128, qs_], sHd[64:128, :], bcd[64:128, :]
            )

        def attn_block(qc, fill=None):
            fill = fill or []
            qs = slice(qc * 512, qc * 512 + 512)
            for pr in range(NPAIR):
                qlo = qT_all[0:64, pr, :]
                qhi = qT_all[64:128, pr, :]
                klo = kT_all[0:64, pr, :]
                khi = kT_all[64:128, pr, :]
                nkt = 4 * qc + 4
                yA = psf.tile([128, 512], F32, tag="ps")
                yB = psf.tile([128, 512], F32, tag="ps")

                def emit_pv(kt, e, first, last, yA=yA, yB=yB, pr=pr, qc=qc):
                    dt = kt - 4 * qc
                    lo = dt * 128 if dt > 0 else 0
                    nc.tensor.matmul(
                        yA[:, lo:512],
                        v_sb[:, kt, pr * VS : pr * VS + 128],
                        e[:, 0, lo:512],
                        start=first,
                        stop=last,
                    )
                    nc.tensor.matmul(
                        yB[:, lo:512],
                        v_sb[:, kt, pr * VS + 128 : pr * VS + 256],
                        e[:, 1, lo:512],
                        start=first,
                        stop=last,
                    )

                def emit_scores(kt, qc=qc, klo=klo, khi=khi, qlo=qlo, qhi=qhi):
                    dt = kt - 4 * qc
                    xlo = dt * 128 if dt > 0 else 0
                    ks = slice(kt * 128, kt * 128 + 128)
                    qw = slice(qc * 512 + xlo, qc * 512 + 512)
                    g = gp.tile([128, 2, 512], F32, tag="g")
                    nc.tensor.matmul(
                        g[:, 0, xlo:512], klo[:, ks], qlo[:, qw],
                        start=True, stop=True,
                    )
                    nc.tensor.matmul(
                        g[:, 1, xlo:512], khi[:, ks], qhi[:, qw],
                        start=True, stop=True,
                    )
                    e = expp.tile([128, 2, 512], BF16, tag="e")
                    if kt % 3 != 1:
                        # true exp on ACT (2/3 of kts: ACT has headroom
                        # and true exp is more accurate than Schraudolph)
                        nc.scalar.activation(
                            e[:, :, xlo:512],
                            g[:, :, xlo:512],
                            mybir.ActivationFunctionType.Exp,
                            scale=0.125,
                        )
                    else:
                        # Schraudolph bit-hack exp on DVE
                        nc.vector.tensor_scalar(
                            e[:, :, xlo:512].bitcast(I16),
                            g[:, :, xlo:512],
                            EXP_A,
                            EXP_B,
                            mybir.AluOpType.mult,
                            mybir.AluOpType.add,
                        )
                    if dt >= 0:
                        # zero the causal triangle (k > q) of the diagonal
                        # block, on the otherwise-idle gpsimd engine
                        bs = slice(dt * 128, dt * 128 + 128)
                        for h in (0, 1):
                            nc.gpsimd.affine_select(
                                out=e[:, h, bs],
                                in_=e[:, h, bs],
                                compare_op=mybir.AluOpType.is_ge,
                                fill=0.0,
                                base=0,
                                pattern=[[1, 128]],
                                channel_multiplier=-1,
                            )
                    return e

                # kt order: the 4 diagonal kts FIRST (their serial
                # exp->mask->PV chain overlaps the pr-start bubble), then
                # the full-width kts, so the pr tail drains without gpsimd
                # masks on the critical path.  Steps of 2: both scores
                # pairs back-to-back on the PE queue, then both lagged PV
                # pairs, halving rhs-stream-switch refills (~105ns each).
                kt_order = list(range(4 * qc, 4 * qc + 4)) + list(range(4 * qc))
                pending = []
                n_emitted = [0]

                def drain_one():
                    kt, e = pending.pop(0)
                    emit_pv(kt, e, n_emitted[0] == 0, n_emitted[0] == nkt - 1)
                    n_emitted[0] += 1

                # diag kts in 2s (their serial exp->mask->PV chain needs
                # early PVs anyway), then full kts in 3s -- the largest run
                # the 3-deep score-psum ring allows.  Longer same-contraction
                # runs amortize the ~105ns LDWEIGHTS row-group-conflict
                # refill paid on each scores<->PV switch.
                batches = [2, 2]
                rem = nkt - 4
                while rem > 4:
                    batches.append(3)
                    rem -= 3
                if rem == 4:
                    batches += [2, 2]
                elif rem > 0:
                    batches.append(rem)
                i0 = 0
                for bi, bsz in enumerate(batches):
                    for kt in kt_order[i0 : i0 + bsz]:
                        pending.append((kt, emit_scores(kt)))
                    # fill the pipeline-fill bubble at pr start with
                    # out-proj chains (pr >= 1: the previous attn block's
                    # pr-3 normalize tail may still be in flight at pr 0),
                    # a reserved chain from two blocks ago, or a projection
                    # chain (dependency-free).
                    if i0 == 0:
                        if pr >= 1:
                            for _ in range(2):
                                if outq:
                                    emit_outproj(*outq.pop(0))
                        elif fill:
                            fill.pop(0)()
                    while len(pending) > 3:
                        drain_one()
                    if i0 == 4 and deferred:
                        # the previous pr's deferred gpsimd scale-multiply:
                        # emitted here so this pr's causal masks (all queued
                        # by i0==4, diag-first) precede it in gpsimd's FIFO
                        pop_deferred()
                    if outq and i0 % 4 == 2 and (pr >= 1 or i0 >= 6):
                        emit_outproj(*outq.pop(0))
                    i0 += bsz
                while pending:
                    drain_one()

                # normalize: y / rowsum (sums_lo on yA partition 64,
                # sums_hi on yB partition 32 -- ones columns of the v
                # stationary blocks).
                if qc == NQC - 1 and pr == NPAIR - 1:
                    # last normalize of the kernel: the final out-proj drain
                    # waits on it.  Avoid the ~5us DRAM-bounce roundtrip:
                    # cast the sums rows to bf16 in place (regular DVE ops
                    # are partition-base safe), broadcast them across
                    # partitions with two K=1 matmuls whose all-ones
                    # stationary sits AT the sums partitions (row positions
                    # 64 / 32), take the reciprocal of the broadcast (base
                    # 0), and scale straight into the slab.  Meanwhile the
                    # PE pre-accumulates ji 0..2 partials of 7 of the 8
                    # final out-proj chains (6 gp + 1 psf accumulators; the
                    # other psf slot holds the broadcast psum), so only one
                    # ji=3 matmul + staging copy + drain per chain remains.
                    while deferred:
                        pop_deferred()
                    sL = sab.tile([128, 512], F32, tag="s", name="sLf")
                    sH = sab.tile([128, 512], F32, tag="s", name="sHf")
                    nc.scalar.copy(sL[0 : D + 1, :], yA[0 : D + 1, :])
                    nc.vector.tensor_copy(sH[64:128, :], yB[64:128, :])
                    nc.vector.tensor_copy(sH[32:33, :], yB[32:33, :])
                    final_chains = [
                        (ti, cc)
                        for ti in range(qc * 4, qc * 4 + 4)
                        for cc in range(2)
                    ]
                    part_accs = []
                    for half in range(3):
                        g2 = gp.tile(
                            [128, 2, 512], F32, tag="g", name=f"pacc2_{half}"
                        )
                        part_accs.append(g2[:, 0, :])
                        part_accs.append(g2[:, 1, :])
                    part_accs.append(
                        psf.tile([128, 512], F32, tag="ps", name="pacc1_0")
                    )

                    def emit_partial(idx):
                        ti, cc = final_chains[idx]
                        tss = slice(ti * 128, ti * 128 + 128)
                        cs = slice(cc * 512, cc * 512 + 512)
                        for ji in range(3):
                            nc.tensor.matmul(
                                part_accs[idx],
                                yTs[ji][:, tss],
                                wp_sb[:, ji, cs],
                                start=(ji == 0),
                                stop=False,
                            )

                    # two partial chains keep the PE busy while the casts run
                    emit_partial(0)
                    emit_partial(1)
                    rcb = bcp.tile([128, 512], BF16, tag="rcb", name="rcb")
                    nc.vector.tensor_copy(rcb[64:65, :], sL[64:65, :])
                    nc.vector.tensor_copy(rcb[32:33, :], sH[32:33, :])
                    bc2 = psf.tile([128, 512], F32, tag="ps", name="bc2")
                    nc.tensor.matmul(
                        bc2[0:64, :], ones_pe[64:65, :], rcb[64:65, :],
                        start=True, stop=True,
                    )
                    nc.tensor.matmul(
                        bc2[64:128, :], ones_pe[32:33, :], rcb[32:33, :],
                        start=True, stop=True,
                    )
                    for idx in range(2, 7):
                        emit_partial(idx)
                    bcr = bcp.tile([128, 512], F32, tag="bc", name="bcrf")
                    nc.vector.reciprocal_approx_fast(bcr[:, :], bc2[:, :])
                    nc.vector.tensor_mul(
                        yTs[pr][0:64, qs], sL[0:64, :], bcr[0:64, :]
                    )
                    nc.gpsimd.tensor_mul(
                        yTs[pr][64:128, qs], sH[64:128, :], bcr[64:128, :]
                    )
                    for idx, (ti, cc) in enumerate(final_chains[:7]):
                        tss = slice(ti * 128, ti * 128 + 128)
                        cs = slice(cc * 512, cc * 512 + 512)
                        nc.tensor.matmul(
                            part_accs[idx],
                            yTs[NPAIR - 1][:, tss],
                            wp_sb[:, NPAIR - 1, cs],
                            start=False,
                            stop=True,
                        )
                        if ti not in o2_tiles:
                            o2_tiles[ti] = outp.tile(
                                [128, C], BF16, tag="o", name=f"o2_{ti}"
                            )
                        o2 = o2_tiles[ti]
                        if idx % 2 == 0:
                            nc.scalar.copy(o2[:, cs], part_accs[idx])
                        else:
                            nc.vector.tensor_copy(o2[:, cs], part_accs[idx])
                        if cc == 1:
                            r0 = ti * 128
                            nc.sync.dma_start(out[r0 : r0 + 48, :], o2[0:48, :])
                            nc.scalar.dma_start(
                                out[r0 + 48 : r0 + 96, :], o2[48:96, :]
                            )
                            nc.gpsimd.dma_start(
                                out[r0 + 96 : r0 + 128, :], o2[96:128, :]
                            )
                            del o2_tiles[ti]
                    emit_outproj(*final_chains[7], drain=True)
                else:
                    # Copies release the yA/yB psum slots the next pr's
                    # first PVs wait on -- split across ACT and DVE.
                    # y_lo + sums_lo live on yA partitions 0-64; sums_hi +
                    # y_hi on yB partitions 62-127 (see the v layout).
                    sL = sab.tile([128, 512], F32, tag="s", name="sL")
                    sH = sab.tile([128, 512], F32, tag="s", name="sH")
                    nc.scalar.copy(sL[0 : D + 1, :], yA[0 : D + 1, :])
                    nc.vector.tensor_copy(sH[64:128, :], yB[64:128, :])
                    nc.vector.tensor_copy(sH[32:33, :], yB[32:33, :])
                    # bounce the raw sums rows through DRAM to broadcast
                    # them across partitions, then ONE in-place reciprocal
                    # on the base-0 [128, 512] broadcast tile (the custom
                    # DVE reciprocal mis-lowers at non-zero partition base)
                    bc = bcp.tile([128, 512], F32, tag="bc")
                    # both stores issue back-to-back, then both broadcast
                    # loads -- a load blocks the queue on its store's
                    # completion, so interleaving would serialize the hops
                    for h, row in ((0, sL[64:65, :]), (1, sH[32:33, :])):
                        nc.sync.dma_start(rcd[pr, qc, h : h + 1, :], row)
                    for h in (0, 1):
                        s = rcd[pr, qc, h, :]
                        src = bass.AP(
                            tensor=s.tensor,
                            offset=s.offset,
                            ap=[[0, 64]] + list(s.ap),
                        )
                        nc.sync.dma_start(bc[64 * h : 64 * h + 64, :], src)
                    bcr = bcp.tile([128, 512], F32, tag="bc", name="bcr")
                    nc.vector.reciprocal_approx_fast(bcr[:, :], bc[:, :])
                    deferred.append((sL, sH, bcr, pr, qs))
                    if qc == 0 and len(deferred) > 1:
                        # qc0's prs are too short for the bounce round-trip:
                        # a gpsimd mul here would still be queued when the
                        # next pr's masks arrive, so use DVE
                        pop_deferred(nc.vector)

                # fill pr-boundary bubbles with projection chains for a
                # later t-chunk (PE-dense, no attention dependencies)
                for _ in range(3):
                    if fill:
                        fill.pop(0)()

            while fill:
                fill.pop(0)()
            # block-end flush runs on DVE: a gpsimd mul here would block the
            # next block's causal masks in gpsimd's strict FIFO
            while deferred:
                pop_deferred(nc.vector)
            if qc != NQC - 1:
                # qc3's chains are emitted inline by the final-normalize path
                for ti in range(qc * 4, qc * 4 + 4):
                    for cc in range(2):
                        outq.append((ti, cc))

        # ---- interleaved schedule ----------------------------------------
        # tch0, tch1, qc0, tch2, qc1, tch3, qc2, qc3: attention qc only
        # needs projection chunks <= qc; projection blocks are PE-bound
        # while attention is ACT/DVE-heavy, so this keeps every engine fed.
        xts = [xt0, None, None, None]

        def load_x(tch, fence_src=None):
            xt = xpool.tile([128, CI, 512], BF16, tag="xt")
            ts_ = slice(tch * 512, tch * 512 + 512)
            if fence_src is not None:
                nc.sync.dma_start(xt[:, 0, 0:2], fence_src)
            nc.sync.dma_start(xt[:], xT_r[:, :, ts_])
            return xt

        xts[1] = load_x(1, fence_src=wk_sb[:, CI - 1, 510:512])
        nc.scalar.dma_start(wp_sb[:, 0, 0:2], xts[1][:, CI - 1, 510:512])
        nc.scalar.dma_start(wp_sb[:], wp_r[:])
        proj_block(0, xts[0])
        xts[2] = load_x(2)
        attn_block(0, fill=proj_chains(1, xts[1]))
        xts[3] = load_x(3)
        attn_block(1, fill=proj_chains(2, xts[2]))
        attn_block(2, fill=proj_chains(3, xts[3]))
        attn_block(3)

        while outq:
            emit_outproj(*outq.pop(0), drain=True)

    nc.finalize()
    return nc


def _get_nc():
    global _CACHED_NC
    if _CACHED_NC is None:
        _CACHED_NC = build_nc()
    return _CACHED_NC


def kernel(x, Wq, Wk, Wv, Wp):
    import ml_dtypes
    from concourse.bass_utils import run_bass_kernel_spmd

    BF = ml_dtypes.bfloat16
    x = np.asarray(x, dtype=np.float32)
    Wq = np.asarray(Wq, dtype=np.float32)
    Wk = np.asarray(Wk, dtype=np.float32)
    Wv = np.asarray(Wv, dtype=np.float32)
    Wp = np.asarray(Wp, dtype=np.float32)

    nc = _get_nc()

    xT = [np.ascontiguousarray(x[b].T).astype(BF) for b in range(B)]
    wqT, wkT, wvT, wpT = [], [], [], []
    for hh in range(2):
        js = slice(JL * hh, JL * hh + JL)
        wqT.append(np.ascontiguousarray(Wq[js, :].T).astype(BF))
        wkT.append(np.ascontiguousarray(Wk[js, :].T).astype(BF))
        wvT.append(np.ascontiguousarray(Wv[js, :].T).astype(BF))
        wpT.append(np.ascontiguousarray(Wp[:, js].T).astype(BF))

    in_maps = []
    for c in range(NCORES):
        b, hh = c // 2, c % 2
        in_maps.append(
            {
                "xT": xT[b],
                "wqT": wqT[hh],
                "wkT": wkT[hh],
                "wvT": wvT[hh],
                "wpT": wpT[hh],
            }
        )

    res = run_bass_kernel_spmd(nc, in_maps, core_ids=list(range(NCORES)))

    out = np.empty((B, T, C), dtype=np.float32)
    for b in range(B):
        out[b] = res.results[2 * b]["out"].astype(np.float32) + res.results[
            2 * b + 1
        ]["out"].astype(np.float32)
    return out



# revision 54
# speedup vs baseline: 1.1890x; 1.1890x over previous
"""Causal self-attention Bass kernel for 8 TRN2 NeuronCores.

Problem: B=4, T=2048, C=1024, H=16 heads, head_dim=64, fp32.
    q = x @ Wq.T ; k = x @ Wk.T ; v = x @ Wv.T          (per head)
    att = softmax(mask(q k^T / 8))
    y = att @ v ; out = y @ Wp.T

Sharding (8 cores): 4-way data parallel over batch x 2-way tensor
parallel over heads. Core c handles batch c//2 and heads 8*(c%2)..+8.
Wq/Wk/Wv column-parallel, Wp row-parallel; the partial outputs of the
two head-halves of each batch are summed on the host (the "all-reduce"
of row-parallel Wp).

Pipeline design (v7):
  - Projections stream xT by 512-wide t-chunks (tch).  The causal
    structure means attention q-chunk qc only needs projections from
    chunks <= qc, so attention blocks INTERLEAVE with projection
    blocks: tch0, tch1, qc0, tch2, qc1, tch3, qc2, qc3.  The
    projection blocks are PE-bound while attention is ACT/DVE-heavy,
    so interleaving keeps every engine fed.
  - exp alternates between ACT (even kt, true exp) and DVE (odd kt,
    Schraudolph bf16 bit-hack: bits = round(s*23.083 + 16248.5) as
    int16 reinterpreted bf16; ~1.8% rms on half the weights, ~5e-3
    added rel err after softmax averaging).
  - PV lags exp by 3 kts; scores/PV emitted in 2-kt batches to halve
    rhs-stream-switch pipeline refills.
  - Diagonal kts run FIRST within each (qc, pr) so the serial
    exp->mask(gpsimd)->PV chain overlaps the pr-start bubble and the
    pr tail drains mask-free.
  - Output projection chains interleave into later attention blocks.
  - Softmax normalize: sums (ones-column of the PV stationary) ->
    ACT/DVE copies release the psum accumulators early -> DVE
    reciprocal -> DRAM-bounce broadcast -> gpsimd scale (deferred one
    pr so gpsimd's FIFO doesn't delay causal masks).
  - Everything bf16: FWL keeps LDWEIGHTS off the critical path, input
    DMA traffic halves.  fp32 accumulation throughout.
"""

from contextlib import ExitStack

import numpy as np

import concourse.bass as bass
import concourse.tile as tile
from concourse import bacc, mybir

F32 = mybir.dt.float32
BF16 = mybir.dt.bfloat16
I16 = mybir.dt.int16

B, T, C, H, D = 4, 2048, 1024, 16, 64
NCORES = 8
JL = 512            # local j dims per core (8 heads * 64)
NPAIR = 4           # local head pairs
CI = C // 128       # 8 c-tiles
NT = T // 128       # 16 t/k tiles
NQC = T // 512      # 4 q chunks
VW = D + 1

# Schraudolph bf16 bit-hack exp: bits = s*0.125*log2(e)*128 + (127*128 - 7.5)
EXP_A = 0.125 * 1.4426950408889634 * 128.0
EXP_B = 127.0 * 128.0 - 7.5

_CACHED_NC = None


def build_nc():
    nc = bacc.Bacc(None)

    xT = nc.dram_tensor("xT", [C, T], BF16, kind="ExternalInput")
    wqT = nc.dram_tensor("wqT", [C, JL], BF16, kind="ExternalInput")
    wkT = nc.dram_tensor("wkT", [C, JL], BF16, kind="ExternalInput")
    wvT = nc.dram_tensor("wvT", [C, JL], BF16, kind="ExternalInput")
    wpT = nc.dram_tensor("wpT", [JL, C], BF16, kind="ExternalInput")
    out = nc.dram_tensor("out", [T, C], BF16, kind="ExternalOutput")
    # bounce buffer for broadcasting softmax reciprocals across partitions
    rcd = nc.dram_tensor("rcd", [NPAIR, NQC, 2, 512], F32)

    xT_r = xT.rearrange("(ci p) t -> p ci t", p=128)
    wq_r = wqT.rearrange("(ci p) j -> p ci j", p=128)
    wk_r = wkT.rearrange("(ci p) j -> p ci j", p=128)
    wv_r = wvT.rearrange("(ci p) j -> p ci j", p=128)
    wp_r = wpT.rearrange("(ji p) c -> p ji c", p=128)

    with tile.TileContext(nc) as tc, ExitStack() as ctx:
        pm = ctx.enter_context(tc.tile_pool(name="pm", bufs=1))
        qkp = ctx.enter_context(tc.tile_pool(name="qkp", bufs=1))
        expp = ctx.enter_context(tc.tile_pool(name="expp", bufs=6))
        bcp = ctx.enter_context(tc.tile_pool(name="bcp", bufs=4))
        # two (sL, sH) f32 staging tiles per normalize, held one pr deep by
        # the deferred scale -> 4 bufs
        sab = ctx.enter_context(tc.tile_pool(name="sab", bufs=4))
        outp = ctx.enter_context(tc.tile_pool(name="outp", bufs=3))
        wpool = ctx.enter_context(tc.tile_pool(name="ph1w", bufs=1))
        xpool = ctx.enter_context(tc.tile_pool(name="ph1x", bufs=2))
        # PSUM: scores (f32, 2 banks each) x3 + shared f32 ring x2 = 8 banks
        gp = ctx.enter_context(tc.tile_pool(name="gp", bufs=3, space="PSUM"))
        psf = ctx.enter_context(tc.tile_pool(name="psf", bufs=2, space="PSUM"))

        # HAM warm-up: the PE clock-gate defaults to 1.2 GHz and only
        # un-throttles after ~3.4us of sustained activity.  The first real
        # matmul can't start until its weights+x DMA lands (~11us), so spend
        # the DMA wait on dummy back-to-back matmuls -- by the time the
        # first projection chain issues, the PE runs at 2.4 GHz.
        warm = pm.tile([128, 128], BF16)
        nc.vector.memset(warm[:], 0.0)
        wacc = psf.tile([128, 512], F32, tag="ps", name="wacc")

        def emit_dummies(n):
            for i in range(n):
                nc.tensor.matmul(
                    wacc[:, 0:128], warm[:], warm[:],
                    start=(i == 0), stop=(i == n - 1),
                )

        emit_dummies(0)

        # v storage per head-pair: two 128-col stationary blocks
        #   lo: [ v_lo(d 0..63) | ones | 0(x63) ]
        #   hi: [ 0(x32) | ones | 0(x31) | v_hi(d 0..63) ]
        # The lo PV matmul puts y_lo on psum partitions 0-63 and sums_lo on
        # partition 64; the hi matmul puts sums_hi on partition 32 and y_hi
        # on partitions 64-127.  With the two heads' outputs on DISJOINT
        # partition halves (and sums on 32-aligned partitions), the
        # psum->SBUF copies and the normalize multiply are partition-
        # aligned, so the scaled output is written straight into the yT
        # slab by ACT/DVE/gpsimd -- no DMA partition-shift bounce.
        # all-ones column tile: rows 64 and 32 serve as the K=1 stationary
        # of the final normalize's broadcast matmuls
        ones_pe = pm.tile([128, 64], BF16)
        nc.vector.memset(ones_pe[:], 1.0)

        VS = 256
        v_sb = pm.tile([128, NT, NPAIR * VS], BF16)
        v_r = v_sb.rearrange("p n (pr c) -> p n pr c", c=VS)
        nc.vector.memset(v_r[:, :, :, 64:192], 0.0)
        nc.vector.memset(v_r[:, :, :, 64:65], 1.0)
        nc.vector.memset(v_r[:, :, :, 160:161], 1.0)

        qT_all = qkp.tile([128, NPAIR, T], BF16, tag="qT_all")
        kT_all = qkp.tile([128, NPAIR, T], BF16, tag="kT_all")
        # one yT tile per head-pair: precise dependency tracking, so an
        # out-proj chain's ji<3 matmuls never wait on a fresher pr's yT
        yTs = [
            qkp.tile([128, T], BF16, tag=f"yT{j}", name=f"yT{j}")
            for j in range(NPAIR)
        ]
        wp_sb = qkp.tile([128, NPAIR, C], BF16, tag="wp")

        wq_sb = wpool.tile([128, CI, JL], BF16, tag="wq")
        wk_sb = wpool.tile([128, CI, JL], BF16, tag="wk")
        wv_sb = wpool.tile([128, CI, JL], BF16, tag="wv")
        # input DMAs in strict consumption order, one stream per HW DGE.
        # The DMA engines fair-share descriptors across all in-flight
        # transfers, so a non-urgent transfer issued early steals bandwidth
        # from the urgent ones.  Queue issue alone does NOT serialize
        # (it's just descriptor enqueue) -- force ordering with tiny fence
        # DMAs that read the tail of the previous transfer and write the
        # head of the next one's buffer (RAW + WAW deps stall the queue
        # until the previous transfer completes).
        xt0 = xpool.tile([128, CI, 512], BF16, tag="xt")
        q = 2
        nc.sync.dma_start(wq_sb[:, 0:q, :], wq_r[:, 0:q, :])
        nc.scalar.dma_start(xt0[:, 0:q, :], xT_r[:, 0:q, 0:512])
        nc.sync.dma_start(wq_sb[:, q:CI, :], wq_r[:, q:CI, :])
        nc.scalar.dma_start(xt0[:, q:CI, :], xT_r[:, q:CI, 0:512])
        nc.sync.dma_start(wk_sb[:, 0, 0:2], wq_sb[:, CI - 1, 510:512])
        nc.sync.dma_start(wk_sb[:], wk_r[:])
        nc.scalar.dma_start(wv_sb[:, 0, 0:2], xt0[:, CI - 1, 510:512])
        nc.scalar.dma_start(wv_sb[:], wv_r[:])
        # wp is issued after xt1's load (fenced on it) -- it isn't needed
        # until the first out-projection (~t+45us) and would otherwise
        # steal DMA bandwidth from xt1 during the startup crunch

        # ---- projection chains for one 512-wide t-chunk -------------------
        def proj_chains(tch, xt, first_mid=None):
            ts_ = slice(tch * 512, tch * 512 + 512)
            chains = []

            def qk_chain(w_sb, dst, eng, pr, mid=None):
                def emit():
                    acc = psf.tile([128, 512], F32, tag="ps", name="pacc")
                    for ci in range(CI):
                        nc.tensor.matmul(
                            acc[:],
                            w_sb[:, ci, pr * 128 : pr * 128 + 128],
                            xt[:, ci, :],
                            start=(ci == 0),
                            stop=(ci == CI - 1),
                        )
                        if mid is not None and ci == 1:
                            # keep the PE clock-gate warm across the
                            # startup DMA stall (ci2..7 land ~4us later)
                            mid()
                    if eng == "v":
                        nc.vector.tensor_copy(dst[:, pr, ts_], acc[:])
                    else:
                        nc.scalar.copy(dst[:, pr, ts_], acc[:])
                return emit

            def v_chain(tl):
                def emit():
                    ti = tch * 4 + tl
                    acc = psf.tile([128, 512], F32, tag="ps", name="pacc")
                    for ci in range(CI):
                        nc.tensor.matmul(
                            acc[:],
                            xt[:, ci, tl * 128 : tl * 128 + 128],
                            wv_sb[:, ci, :],
                            start=(ci == 0),
                            stop=(ci == CI - 1),
                        )
                    ar = acc[:].rearrange(
                        "p (pr two d) -> p pr two d", two=2, d=D
                    )
                    nc.vector.tensor_copy(
                        v_r[:, ti, :, 0:64], ar[:, :, 0, :]
                    )
                    nc.vector.tensor_copy(
                        v_r[:, ti, :, 192:256], ar[:, :, 1, :]
                    )
                return emit

            for pr in range(NPAIR):
                chains.append(
                    qk_chain(
                        wq_sb, qT_all, "v", pr,
                        mid=first_mid if pr == 0 else None,
                    )
                )
            for pr in range(NPAIR):
                chains.append(qk_chain(wk_sb, kT_all, "a", pr))
            for tl in range(4):
                chains.append(v_chain(tl))
            return chains

        def proj_block(tch, xt, first_mid=None):
            for emit in proj_chains(tch, xt, first_mid=first_mid):
                emit()

        # ---- attention machinery -----------------------------------------
        outq = []          # pending out-proj (ti, cc) chains
        n_chain = [0]
        o2_tiles = {}      # ti -> [128, 1024] staging tile (cc=0 allocates)

        def emit_outproj(ti, cc, drain=False):
            tss = slice(ti * 128, ti * 128 + 128)
            cs = slice(cc * 512, cc * 512 + 512)
            acc2 = gp.tile([128, 2, 512], F32, tag="g")
            acc = acc2[:, 0, :]
            for ji in range(NPAIR):
                nc.tensor.matmul(
                    acc,
                    yTs[ji][:, tss],
                    wp_sb[:, ji, cs],
                    start=(ji == 0),
                    stop=(ji == NPAIR - 1),
                )
            # stage both cc halves of a ti into one [128, 1024] tile so the
            # out DMA moves 2KB-contiguous rows (efficient descriptors)
            if ti not in o2_tiles:
                o2_tiles[ti] = outp.tile([128, C], BF16, tag="o", name=f"o2_{ti}")
            o2 = o2_tiles[ti]
            if n_chain[0] % 2 == 0:
                nc.vector.tensor_copy(o2[:, cs], acc)
            else:
                nc.scalar.copy(o2[:, cs], acc)
            n_chain[0] += 1
            if cc == 1:
                # split rows across DGEs so the transfer drains in parallel;
                # the final drain also enlists gpsimd's SWDGE
                r0 = ti * 128
                if drain:
                    nc.sync.dma_start(out[r0 : r0 + 48, :], o2[0:48, :])
                    nc.scalar.dma_start(out[r0 + 48 : r0 + 96, :], o2[48:96, :])
                    nc.gpsimd.dma_start(out[r0 + 96 : r0 + 128, :], o2[96:128, :])
                else:
                    # mid-kernel: keep the issue off ACT (it carries exp +
                    # copies); the sync DGE has headroom here
                    nc.sync.dma_start(out[r0 : r0 + 64, :], o2[0:64, :])
                    nc.sync.dma_start(out[r0 + 64 : r0 + 128, :], o2[64:128, :])
                del o2_tiles[ti]

        # deferred normalize tail: the gpsimd scale-multiplies of (qc, pr)
        # are emitted after (qc, pr+1)'s masks so gpsimd's strict FIFO
        # doesn't delay the causal-mask affine_selects.  The muls write the
        # scaled attention output DIRECTLY into the yT slab (partition-
        # aligned thanks to the v layout) -- no DMA bounce.
        deferred = []

        def pop_deferred(eng=None):
            # lo half on gpsimd (partition base 0 -- the only base gpsimd
            # handles reliably), hi half on DVE
            sLd, sHd, bcd, pr_, qs_ = deferred.pop(0)
            (eng or nc.gpsimd).tensor_mul(
                yTs[pr_][0:64, qs_], sLd[0:64, :], bcd[0:64, :]
            )
            (eng or nc.gpsimd).tensor_mul(
                yTs[pr_][64:128, qs_], sHd[64:128, :], bcd[64:128, :]
            )

        def attn_block(qc, fill=None):
            fill = fill or []
            qs = slice(qc * 512, qc * 512 + 512)
            for pr in range(NPAIR):
                qlo = qT_all[0:64, pr, :]
                qhi = qT_all[64:128, pr, :]
                klo = kT_all[0:64, pr, :]
                khi = kT_all[64:128, pr, :]
                nkt = 4 * qc + 4
                yA = psf.tile([128, 512], F32, tag="ps")
                yB = psf.tile([128, 512], F32, tag="ps")

                def emit_pv(kt, e, first, last, yA=yA, yB=yB, pr=pr, qc=qc):
                    dt = kt - 4 * qc
                    lo = dt * 128 if dt > 0 else 0
                    nc.tensor.matmul(
                        yA[:, lo:512],
                        v_sb[:, kt, pr * VS : pr * VS + 128],
                        e[:, 0, lo:512],
                        start=first,
                        stop=last,
                    )
                    nc.tensor.matmul(
                        yB[:, lo:512],
                        v_sb[:, kt, pr * VS + 128 : pr * VS + 256],
                        e[:, 1, lo:512],
                        start=first,
                        stop=last,
                    )

                def emit_scores(kt, qc=qc, klo=klo, khi=khi, qlo=qlo, qhi=qhi):
                    dt = kt - 4 * qc
                    xlo = dt * 128 if dt > 0 else 0
                    ks = slice(kt * 128, kt * 128 + 128)
                    qw = slice(qc * 512 + xlo, qc * 512 + 512)
                    g = gp.tile([128, 2, 512], F32, tag="g")
                    nc.tensor.matmul(
                        g[:, 0, xlo:512], klo[:, ks], qlo[:, qw],
                        start=True, stop=True,
                    )
                    nc.tensor.matmul(
                        g[:, 1, xlo:512], khi[:, ks], qhi[:, qw],
                        start=True, stop=True,
                    )
                    e = expp.tile([128, 2, 512], BF16, tag="e")
                    if kt % 3 != 1:
                        # true exp on ACT (2/3 of kts: ACT has headroom
                        # and true exp is more accurate than Schraudolph)
                        nc.scalar.activation(
                            e[:, :, xlo:512],
                            g[:, :, xlo:512],
                            mybir.ActivationFunctionType.Exp,
                            scale=0.125,
                        )
                    else:
                        # Schraudolph bit-hack exp on DVE
                        nc.vector.tensor_scalar(
                            e[:, :, xlo:512].bitcast(I16),
                            g[:, :, xlo:512],
                            EXP_A,
                            EXP_B,
                            mybir.AluOpType.mult,
                            mybir.AluOpType.add,
                        )
                    if dt >= 0:
                        # zero the causal triangle (k > q) of the diagonal
                        # block, on the otherwise-idle gpsimd engine
                        bs = slice(dt * 128, dt * 128 + 128)
                        for h in (0, 1):
                            nc.gpsimd.affine_select(
                                out=e[:, h, bs],
                                in_=e[:, h, bs],
                                compare_op=mybir.AluOpType.is_ge,
                                fill=0.0,
                                base=0,
                                pattern=[[1, 128]],
                                channel_multiplier=-1,
                            )
                    return e

                # kt order: the 4 diagonal kts FIRST (their serial
                # exp->mask->PV chain overlaps the pr-start bubble), then
                # the full-width kts, so the pr tail drains without gpsimd
                # masks on the critical path.  Steps of 2: both scores
                # pairs back-to-back on the PE queue, then both lagged PV
                # pairs, halving rhs-stream-switch refills (~105ns each).
                kt_order = list(range(4 * qc, 4 * qc + 4)) + list(range(4 * qc))
                pending = []
                n_emitted = [0]

                def drain_one():
                    kt, e = pending.pop(0)
                    emit_pv(kt, e, n_emitted[0] == 0, n_emitted[0] == nkt - 1)
                    n_emitted[0] += 1

                # diag kts in 2s (their serial exp->mask->PV chain needs
                # early PVs anyway), then full kts in 3s -- the largest run
                # the 3-deep score-psum ring allows.  Longer same-contraction
                # runs amortize the ~105ns LDWEIGHTS row-group-conflict
                # refill paid on each scores<->PV switch.
                batches = [2, 2]
                rem = nkt - 4
                while rem > 4:
                    batches.append(3)
                    rem -= 3
                if rem == 4:
                    batches += [2, 2]
                elif rem > 0:
                    batches.append(rem)
                i0 = 0
                for bi, bsz in enumerate(batches):
                    for kt in kt_order[i0 : i0 + bsz]:
                        pending.append((kt, emit_scores(kt)))
                    # fill the pipeline-fill bubble at pr start with
                    # out-proj chains (pr >= 1: the previous attn block's
                    # pr-3 normalize tail may still be in flight at pr 0),
                    # a reserved chain from two blocks ago, or a projection
                    # chain (dependency-free).
                    if i0 == 0:
                        if pr >= 1:
                            for _ in range(2):
                                if outq:
                                    emit_outproj(*outq.pop(0))
                        elif fill:
                            fill.pop(0)()
                    while len(pending) > 3:
                        drain_one()
                    if i0 == 4 and deferred:
                        # the previous pr's deferred gpsimd scale-multiply:
                        # emitted here so this pr's causal masks (all queued
                        # by i0==4, diag-first) precede it in gpsimd's FIFO
                        pop_deferred()
                    if outq and i0 % 4 == 2 and (pr >= 1 or i0 >= 6):
                        emit_outproj(*outq.pop(0))
                    i0 += bsz
                while pending:
                    drain_one()

                # normalize: y / rowsum (sums_lo on yA partition 64,
                # sums_hi on yB partition 32 -- ones columns of the v
                # stationary blocks).
                if qc == NQC - 1 and pr == NPAIR - 1:
                    # last normalize of the kernel: the final out-proj drain
                    # waits on it.  Avoid the ~5us DRAM-bounce roundtrip:
                    # cast the sums rows to bf16 in place (regular DVE ops
                    # are partition-base safe), broadcast them across
                    # partitions with two K=1 matmuls whose all-ones
                    # stationary sits AT the sums partitions (row positions
                    # 64 / 32), take the reciprocal of the broadcast (base
                    # 0), and scale straight into the slab.  Meanwhile the
                    # PE pre-accumulates ji 0..2 partials of 7 of the 8
                    # final out-proj chains (6 gp + 1 psf accumulators; the
                    # other psf slot holds the broadcast psum), so only one
                    # ji=3 matmul + staging copy + drain per chain remains.
                    while deferred:
                        pop_deferred()
                    sL = sab.tile([128, 512], F32, tag="s", name="sLf")
                    sH = sab.tile([128, 512], F32, tag="s", name="sHf")
                    nc.scalar.copy(sL[0 : D + 1, :], yA[0 : D + 1, :])
                    nc.vector.tensor_copy(sH[64:128, :], yB[64:128, :])
                    nc.vector.tensor_copy(sH[32:33, :], yB[32:33, :])
                    final_chains = [
                        (ti, cc)
                        for ti in range(qc * 4, qc * 4 + 4)
                        for cc in range(2)
                    ]
                    part_accs = []
                    for half in range(3):
                        g2 = gp.tile(
                            [128, 2, 512], F32, tag="g", name=f"pacc2_{half}"
                        )
                        part_accs.append(g2[:, 0, :])
                        part_accs.append(g2[:, 1, :])
                    part_accs.append(
                        psf.tile([128, 512], F32, tag="ps", name="pacc1_0")
                    )

                    def emit_partial(idx):
                        ti, cc = final_chains[idx]
                        tss = slice(ti * 128, ti * 128 + 128)
                        cs = slice(cc * 512, cc * 512 + 512)
                        for ji in range(3):
                            nc.tensor.matmul(
                                part_accs[idx],
                                yTs[ji][:, tss],
                                wp_sb[:, ji, cs],
                                start=(ji == 0),
                                stop=False,
                            )

                    # two partial chains keep the PE busy while the casts run
                    emit_partial(0)
                    emit_partial(1)
                    rcb = bcp.tile([128, 512], BF16, tag="rcb", name="rcb")
                    nc.vector.tensor_copy(rcb[64:65, :], sL[64:65, :])
                    nc.vector.tensor_copy(rcb[32:33, :], sH[32:33, :])
                    bc2 = psf.tile([128, 512], F32, tag="ps", name="bc2")
                    nc.tensor.matmul(
                        bc2[0:64, :], ones_pe[64:65, :], rcb[64:65, :],
                        start=True, stop=True,
                    )
                    nc.tensor.matmul(
                        bc2[64:128, :], ones_pe[32:33, :], rcb[32:33, :],
                        start=True, stop=True,
                    )
                    for idx in range(2, 7):
                        emit_partial(idx)
                    bcr = bcp.tile([128, 512], F32, tag="bc", name="bcrf")
                    nc.vector.reciprocal_approx_fast(bcr[:, :], bc2[:, :])
                    nc.vector.tensor_mul(
                        yTs[pr][0:64, qs], sL[0:64, :], bcr[0:64, :]
                    )
                    nc.gpsimd.tensor_mul(
                        yTs[pr][64:128, qs], sH[64:128, :], bcr[64:128, :]
                    )
                    for idx, (ti, cc) in enumerate(final_chains[:7]):
                        tss = slice(ti * 128, ti * 128 + 128)
                        cs = slice(cc * 512, cc * 512 + 512)
                        nc.tensor.matmul(
                            part_accs[idx],
                            yTs[NPAIR - 1][:, tss],
                            wp_sb[:, NPAIR - 1, cs],
                            start=False,
                            stop=True,
                        )
                        if ti not in o2_tiles:
                            o2_tiles[ti] = outp.tile(
                                [128, C], BF16, tag="o", name=f"o2_{ti}"
                            )
                        o2 = o2_tiles[ti]
                        if idx % 2 == 0:
                            nc.scalar.copy(o2[:, cs], part_accs[idx])
                        else:
                            nc.vector.tensor_copy(o2[:, cs], part_accs[idx])
                        if cc == 1:
                            r0 = ti * 128
                            nc.sync.dma_start(out[r0 : r0 + 48, :], o2[0:48, :])
                            nc.scalar.dma_start(
                                out[r0 + 48 : r0 + 96, :], o2[48:96, :]
                            )
                            nc.gpsimd.dma_start(
                                out[r0 + 96 : r0 + 128, :], o2[96:128, :]
                            )
                            del o2_tiles[ti]
                    emit_outproj(*final_chains[7], drain=True)
                else:
                    # Copies release the yA/yB psum slots the next pr's
                    # first PVs wait on -- split across ACT and DVE.
                    # y_lo + sums_lo live on yA partitions 0-64; sums_hi +
                    # y_hi on yB partitions 62-127 (see the v layout).
                    sL = sab.tile([128, 512], F32, tag="s", name="sL")
                    sH = sab.tile([128, 512], F32, tag="s", name="sH")
                    nc.scalar.copy(sL[0 : D + 1, :], yA[0 : D + 1, :])
                    nc.vector.tensor_copy(sH[64:128, :], yB[64:128, :])
                    nc.vector.tensor_copy(sH[32:33, :], yB[32:33, :])
                    # bounce the raw sums rows through DRAM to broadcast
                    # them across partitions, then ONE in-place reciprocal
                    # on the base-0 [128, 512] broadcast tile (the custom
                    # DVE reciprocal mis-lowers at non-zero partition base)
                    bc = bcp.tile([128, 512], F32, tag="bc")
                    # both stores issue back-to-back, then both broadcast
                    # loads -- a load blocks the queue on its store's
                    # completion, so interleaving would serialize the hops
                    for h, row in ((0, sL[64:65, :]), (1, sH[32:33, :])):
                        nc.sync.dma_start(rcd[pr, qc, h : h + 1, :], row)
                    for h in (0, 1):
                        s = rcd[pr, qc, h, :]
                        src = bass.AP(
                            tensor=s.tensor,
                            offset=s.offset,
                            ap=[[0, 64]] + list(s.ap),
                        )
                        nc.sync.dma_start(bc[64 * h : 64 * h + 64, :], src)
                    bcr = bcp.tile([128, 512], F32, tag="bc", name="bcr")
                    nc.vector.reciprocal_approx_fast(bcr[:, :], bc[:, :])
                    deferred.append((sL, sH, bcr, pr, qs))
                    if qc == 0 and len(deferred) > 1:
                        # qc0's prs are too short for the bounce round-trip:
                        # a gpsimd mul here would still be queued when the
                        # next pr's masks arrive, so use DVE
                        pop_deferred(nc.vector)

                # fill pr-boundary bubbles with projection chains for a
                # later t-chunk (PE-dense, no attention dependencies)
                for _ in range(3):
                    if fill:
                        fill.pop(0)()

            while fill:
                fill.pop(0)()
            # block-end flush runs on DVE: a gpsimd mul here would block the
            # next block's causal masks in gpsimd's strict FIFO
            while deferred:
                pop_deferred(nc.vector)
            if qc != NQC - 1:
                # qc3's chains are emitted inline by the final-normalize path
                for ti in range(qc * 4, qc * 4 + 4):
                    for cc in range(2):
                        outq.append((ti, cc))

        # ---- interleaved schedule ----------------------------------------
        # tch0, tch1, qc0, tch2, qc1, tch3, qc2, qc3: attention qc only
        # needs projection chunks <= qc; projection blocks are PE-bound
        # while attention is ACT/DVE-heavy, so this keeps every engine fed.
        xts = [xt0, None, None, None]

        def load_x(tch, fence_src=None):
            xt = xpool.tile([128, CI, 512], BF16, tag="xt")
            ts_ = slice(tch * 512, tch * 512 + 512)
            if fence_src is not None:
                nc.sync.dma_start(xt[:, 0, 0:2], fence_src)
            nc.sync.dma_start(xt[:], xT_r[:, :, ts_])
            return xt

        xts[1] = load_x(1, fence_src=wk_sb[:, CI - 1, 510:512])
        nc.scalar.dma_start(wp_sb[:, 0, 0:2], xts[1][:, CI - 1, 510:512])
        nc.scalar.dma_start(wp_sb[:], wp_r[:])
        proj_block(0, xts[0])
        xts[2] = load_x(2)
        attn_block(0, fill=proj_chains(1, xts[1]))
        xts[3] = load_x(3)
        attn_block(1, fill=proj_chains(2, xts[2]))
        attn_block(2, fill=proj_chains(3, xts[3]))
        attn_block(3)

        while outq:
            emit_outproj(*outq.pop(0), drain=True)

    nc.finalize()
    return nc


def _get_nc():
    global _CACHED_NC
    if _CACHED_NC is None:
        _CACHED_NC = build_nc()
    return _CACHED_NC


def kernel(x, Wq, Wk, Wv, Wp):
    import ml_dtypes
    from concourse.bass_utils import run_bass_kernel_spmd

    BF = ml_dtypes.bfloat16
    x = np.asarray(x, dtype=np.float32)
    Wq = np.asarray(Wq, dtype=np.float32)
    Wk = np.asarray(Wk, dtype=np.float32)
    Wv = np.asarray(Wv, dtype=np.float32)
    Wp = np.asarray(Wp, dtype=np.float32)

    nc = _get_nc()

    xT = [np.ascontiguousarray(x[b].T).astype(BF) for b in range(B)]
    wqT, wkT, wvT, wpT = [], [], [], []
    for hh in range(2):
        js = slice(JL * hh, JL * hh + JL)
        wqT.append(np.ascontiguousarray(Wq[js, :].T).astype(BF))
        wkT.append(np.ascontiguousarray(Wk[js, :].T).astype(BF))
        wvT.append(np.ascontiguousarray(Wv[js, :].T).astype(BF))
        wpT.append(np.ascontiguousarray(Wp[:, js].T).astype(BF))

    in_maps = []
    for c in range(NCORES):
        b, hh = c // 2, c % 2
        in_maps.append(
            {
                "xT": xT[b],
                "wqT": wqT[hh],
                "wkT": wkT[hh],
                "wvT": wvT[hh],
                "wpT": wpT[hh],
            }
        )

    res = run_bass_kernel_spmd(nc, in_maps, core_ids=list(range(NCORES)))

    out = np.empty((B, T, C), dtype=np.float32)
    for b in range(B):
        out[b] = res.results[2 * b]["out"].astype(np.float32) + res.results[
            2 * b + 1
        ]["out"].astype(np.float32)
    return out

